# revision 1
# baseline (speedup 1.0000x reference)
"""Trainium2 Bass kernel for nn_Connection_v5 (geodesic-spray-style RHS).

Math (per sample n, D=128, 2D=256):
    x = input_[:, :D], v = input_[:, D:]
    z1 = x @ W1.T + b1            [2D]
    h  = relu(z1), mask = z1 > 0  [2D]
    s  = sigmoid(h @ W2.T + b2)   [D]
    sign_j = -1 if j < 4 else 1
    g  = (s + 0.618) * sign;  jac[i,j] = sign_i s_i(1-s_i) * (W2 (mask*W1))[i,j]
    dv[j] = -1/g_j * sum_i v_i^2 jac[i,j] + 2 v_j / g_j * sum_i v_i jac[j,i]
    out = [v, dv]

Folded form used here (signs/scales pushed into host-precomputed weights):
    nsps = (s-1)*s            (= -s(1-s))
    gr   = 1/(s+0.618)
    u    = v @ W1.T                       ; mu = mask * u
    wt   = v^2 * nsps                     ; at = wt @ (sign_i*W2) ; am = mask * at
    At   = am @ (W1*sign_j)               ; Ct = mu @ (-2*W2.T)
    dv   = gr*At + (v*nsps*gr)*Ct

Sharding: pure data-parallel over N=8192 across 8 cores (1024 rows each);
weights replicated. On-chip layout is feature-major [feat, n]; sample-major
<->feature-major conversion via PE transposes with an identity matrix.
Precision: M1 (z1, decides the relu mask) in full fp32; the other five
matmuls in bf16 (fp32 PSUM accumulate); final combine in fp32.
"""

import os
import numpy as np

D = 128
TWO_D = 256
N_TOTAL = 8192
NCORES = 8
N_CORE = N_TOTAL // NCORES  # 1024
NF = 256                    # samples per pipeline chunk (matmul moving dim)
CONST = 0.618
SIGN = 4

_CACHE = {}


def _build(n_core=N_CORE):
    """Build + compile the per-core Bass module (cached)."""
    from contextlib import ExitStack

    import concourse.bacc as bacc
    import concourse.mybir as mybir
    import concourse.tile as tile

    f32 = mybir.dt.float32
    bf16 = mybir.dt.bfloat16
    Act = mybir.ActivationFunctionType
    Op = mybir.AluOpType

    nchunk = n_core // NF
    nb = NF // 128  # 128-row blocks per chunk

    nc = bacc.Bacc("TRN2", target_bir_lowering=False, debug=False,
                   num_devices=NCORES)

    inp = nc.dram_tensor("inp", [n_core, TWO_D], f32, kind="ExternalInput").ap()
    w1t = nc.dram_tensor("w1t", [D, TWO_D], f32, kind="ExternalInput").ap()
    w1tb = nc.dram_tensor("w1tb", [D, TWO_D], bf16, kind="ExternalInput").ap()
    w2t = nc.dram_tensor("w2t", [TWO_D, D], bf16, kind="ExternalInput").ap()
    w2sgn = nc.dram_tensor("w2sgn", [D, TWO_D], bf16,
                           kind="ExternalInput").ap()
    w1sgn = nc.dram_tensor("w1sgn", [TWO_D, D], bf16,
                           kind="ExternalInput").ap()
    w2t2 = nc.dram_tensor("w2t2", [TWO_D, D], bf16, kind="ExternalInput").ap()
    b1d = nc.dram_tensor("b1d", [D, 2], f32, kind="ExternalInput").ap()
    b2d = nc.dram_tensor("b2d", [D, 1], f32, kind="ExternalInput").ap()
    idn = nc.dram_tensor("idn", [128, 128], f32, kind="ExternalInput").ap()
    out = nc.dram_tensor("out", [n_core, TWO_D], f32, kind="ExternalOutput").ap()

    with tile.TileContext(nc) as tc:
        with ExitStack() as ctx:
            singles = ctx.enter_context(tc.tile_pool(name="singles", bufs=1))
            io = ctx.enter_context(tc.tile_pool(name="io", bufs=3))
            acts = ctx.enter_context(tc.tile_pool(name="acts", bufs=3))
            psum = ctx.enter_context(
                tc.tile_pool(name="psum", bufs=8, space="PSUM"))

            # identity first (the transposes need it immediately); all other
            # weights go via SWDGE (gpsimd) so the Sync HWDGE queue is free
            # for the first input tiles.
            sb_id = singles.tile([128, 128], f32, name="sb_id")
            nc.sync.dma_start(out=sb_id, in_=idn)
            sb_b1 = singles.tile([128, 2], f32, name="sb_b1")
            nc.gpsimd.dma_start(out=sb_b1, in_=b1d)
            sb_b2 = singles.tile([128, 1], f32, name="sb_b2")
            nc.gpsimd.dma_start(out=sb_b2, in_=b2d)
            # prime the ACT function tables (Relu/Sigmoid/Copy) with dummy
            # [128,1] ops so the ~1.3us ACT_TABLE_LOADs overlap the DMAs
            # instead of blocking the first real activation.
            warm = singles.tile([128, 1], f32, name="warm")
            nc.scalar.activation(out=warm, in_=sb_id[:, 0:1],
                                 func=Act.Relu, bias=sb_b2[:, 0:1], scale=1.0)
            nc.scalar.activation(out=warm, in_=sb_id[:, 0:1],
                                 func=Act.Sigmoid, bias=sb_b2[:, 0:1],
                                 scale=1.0)
            sb_w1t = singles.tile([128, TWO_D], f32, name="sb_w1t")
            nc.gpsimd.dma_start(out=sb_w1t, in_=w1t)
            sb_w1tb = singles.tile([128, TWO_D], bf16, name="sb_w1tb")
            nc.gpsimd.dma_start(out=sb_w1tb, in_=w1tb)
            sb_w2t = singles.tile([128, 2, D], bf16, name="sb_w2t")
            nc.gpsimd.dma_start(out=sb_w2t,
                                in_=w2t.rearrange("(c p) m -> p c m", p=128))
            sb_w2sgn = singles.tile([128, TWO_D], bf16, name="sb_w2sgn")
            nc.gpsimd.dma_start(out=sb_w2sgn, in_=w2sgn)
            sb_w1sgn = singles.tile([128, 2, D], bf16, name="sb_w1sgn")
            nc.gpsimd.dma_start(out=sb_w1sgn,
                                in_=w1sgn.rearrange("(c p) m -> p c m", p=128))
            sb_w2t2 = singles.tile([128, 2, D], bf16, name="sb_w2t2")
            nc.gpsimd.dma_start(out=sb_w2t2,
                                in_=w2t2.rearrange("(c p) m -> p c m", p=128))

            inp_v = inp.rearrange("(c b p) d -> c p b d", p=128, b=nb)
            outd_v = out[:, D:TWO_D].rearrange("(c b p) d -> c p b d",
                                               p=128, b=nb)

            # v passthrough: one bulk DRAM->DRAM copy, independent of all
            # compute; overlaps with everything.
            nc.sync.dma_start(out=out[:, 0:D], in_=inp[:, D:TWO_D])

            # Two-stage software pipeline: front(c) produces the s-chain and
            # the feature-major operands; back(c) runs the second-order
            # matmuls and the combine. Emitting front(c+1) before back(c)
            # gives every engine chunk-independent work to overlap.
            state = {}

            def front(c):
                inb = io.tile([128, nb, TWO_D], f32, tag="inb", name="inb")
                nc.sync.dma_start(out=inb, in_=inp_v[c])

                # sample-major -> feature-major via PE transposes
                ps_tr = psum.tile([128, 2, NF], f32, tag="ps", name="ps_tr")
                for b in range(nb):
                    nc.tensor.transpose(ps_tr[:, 0, 128 * b:128 * (b + 1)],
                                        inb[:, b, 0:D], sb_id)
                    nc.tensor.transpose(ps_tr[:, 1, 128 * b:128 * (b + 1)],
                                        inb[:, b, D:TWO_D], sb_id)
                xv = acts.tile([128, 2, NF], f32, tag="xv", name="xv")
                nc.scalar.copy(out=xv, in_=ps_tr)
                xT = xv[:, 0, :]
                vT = xv[:, 1, :]
                # bf16 copy of vT for the M3 matmul (cast during PSUM drain)
                vTb = acts.tile([128, NF], bf16, tag="vTb", name="vTb")
                nc.scalar.copy(out=vTb, in_=ps_tr[:, 1, :])

                # M1: z1^T = W1 @ x^T (full fp32: mask depends on its sign)
                ps_z1 = psum.tile([128, 2, NF], f32, tag="ps", name="ps_z1")
                for k in range(2):
                    nc.tensor.matmul(ps_z1[:, k, :],
                                     sb_w1t[:, 128 * k:128 * (k + 1)], xT,
                                     start=True, stop=True)
                h = acts.tile([128, 2, NF], bf16, tag="h", name="h")
                for k in range(2):
                    nc.scalar.activation(out=h[:, k, :], in_=ps_z1[:, k, :],
                                         func=Act.Relu,
                                         bias=sb_b1[:, k:k + 1], scale=1.0)

                # M2: z2 = W2 @ h (accumulate over the two k-chunks)
                ps_z2 = psum.tile([128, NF], f32, tag="ps", name="ps_z2")
                for k in range(2):
                    nc.tensor.matmul(ps_z2, sb_w2t[:, k, :], h[:, k, :],
                                     start=(k == 0), stop=(k == 1))
                s = acts.tile([128, NF], f32, tag="s", name="s")
                nc.scalar.activation(out=s, in_=ps_z2, func=Act.Sigmoid,
                                     bias=sb_b2[:, 0:1], scale=1.0)

                gs = acts.tile([128, NF], f32, tag="gs", name="gs")
                nc.vector.tensor_scalar_add(gs, s, CONST)
                gr = acts.tile([128, NF], f32, tag="gr", name="gr")
                nc.vector.reciprocal_approx_fast(out=gr, in_=gs)
                nsps = acts.tile([128, NF], f32, tag="nsps", name="nsps")
                nc.vector.scalar_tensor_tensor(out=nsps, in0=s, scalar=-1.0,
                                               in1=s, op0=Op.add, op1=Op.mult)
                v2 = acts.tile([128, NF], f32, tag="v2", name="v2")
                nc.gpsimd.tensor_tensor(v2, vT, vT, Op.mult)
                state[c] = dict(vT=vT, vTb=vTb, h=h, gr=gr, nsps=nsps, v2=v2)

            def backA(c):
                """Second-order ops whose inputs are ready as soon as
                front(c) is done — emitted at the START of the next step so
                every engine leads with runnable work."""
                st = state[c]
                vT, vTb, h = st["vT"], st["vTb"], st["h"]
                gr, nsps, v2 = st["gr"], st["nsps"], st["v2"]

                wt = acts.tile([128, NF], bf16, tag="wt", name="wt")
                nc.vector.tensor_tensor(wt, v2, nsps, Op.mult)
                qt = acts.tile([128, NF], f32, tag="qt", name="qt")
                nc.vector.tensor_tensor(qt, nsps, gr, Op.mult)
                vq = acts.tile([128, NF], f32, tag="vq", name="vq")
                nc.gpsimd.tensor_tensor(vq, vT, qt, Op.mult)

                # M3: u^T = W1 @ v^T (bf16)
                ps_u = psum.tile([128, 2, NF], f32, tag="ps", name="ps_u")
                for k in range(2):
                    nc.tensor.matmul(ps_u[:, k, :],
                                     sb_w1tb[:, 128 * k:128 * (k + 1)],
                                     vTb, start=True, stop=True)
                # M4: at^T, contraction over i with (sign_i*W2)
                ps_a = psum.tile([128, 2, NF], f32, tag="ps", name="ps_a")
                for k in range(2):
                    nc.tensor.matmul(ps_a[:, k, :],
                                     sb_w2sgn[:, 128 * k:128 * (k + 1)],
                                     wt, start=True, stop=True)

                # mask-mul drains: mu = (h>0)*u, am = (h>0)*at
                mu = acts.tile([128, 2, NF], bf16, tag="mu", name="mu")
                am = acts.tile([128, 2, NF], bf16, tag="am", name="am")
                nc.vector.scalar_tensor_tensor(
                    out=mu, in0=h, scalar=0.0, in1=ps_u,
                    op0=Op.is_gt, op1=Op.mult)
                nc.vector.scalar_tensor_tensor(
                    out=am, in0=h, scalar=0.0, in1=ps_a,
                    op0=Op.is_gt, op1=Op.mult)
                st.update(mu=mu, am=am, vq=vq)

            def backB(c):
                st = state.pop(c)
                gr, vq, mu, am = st["gr"], st["vq"], st["mu"], st["am"]

                # M5: At = am @ (W1*sign_j);  M6: Ct = mu @ (-2*W2.T)
                ps_AC = psum.tile([128, 2, NF], f32, tag="ps", name="ps_AC")
                for k in range(2):
                    nc.tensor.matmul(ps_AC[:, 0, :], sb_w1sgn[:, k, :],
                                     am[:, k, :],
                                     start=(k == 0), stop=(k == 1))
                for k in range(2):
                    nc.tensor.matmul(ps_AC[:, 1, :], sb_w2t2[:, k, :],
                                     mu[:, k, :],
                                     start=(k == 0), stop=(k == 1))

                rA = acts.tile([128, NF], f32, tag="rA", name="rA")
                nc.vector.tensor_tensor(rA, gr, ps_AC[:, 0, :], Op.mult)
                t2 = acts.tile([128, NF], f32, tag="t2", name="t2")
                nc.vector.tensor_tensor(t2, vq, ps_AC[:, 1, :], Op.mult)
                dvT = acts.tile([128, NF], f32, tag="dvT", name="dvT")
                nc.vector.tensor_tensor(dvT, rA, t2, Op.add)

                # feature-major -> sample-major and store
                ps_dv = psum.tile([128, NF], f32, tag="ps", name="ps_dv")
                for b in range(nb):
                    nc.tensor.transpose(ps_dv[:, 128 * b:128 * (b + 1)],
                                        dvT[:, 128 * b:128 * (b + 1)], sb_id)
                ob = io.tile([128, nb, D], f32, tag="ob", name="ob")
                nc.scalar.copy(out=ob, in_=ps_dv.rearrange(
                    "p (b d) -> p b d", b=nb))
                nc.sync.dma_start(out=outd_v[c], in_=ob)

            for c in range(nchunk):
                if c > 0:
                    backA(c - 1)
                front(c)
                if c > 0:
                    backB(c - 1)
            backA(nchunk - 1)
            backB(nchunk - 1)

    nc.compile()
    return nc


def _get_nc(n_core=N_CORE):
    key = ("nc", n_core)
    if key not in _CACHE:
        _CACHE[key] = _build(n_core)
    return _CACHE[key]


def _host_weights(W1, b1, W2, b2):
    import ml_dtypes

    W1 = np.asarray(W1, np.float32)
    b1 = np.asarray(b1, np.float32)
    W2 = np.asarray(W2, np.float32)
    b2 = np.asarray(b2, np.float32)
    bf16 = ml_dtypes.bfloat16
    sign = np.where(np.arange(D) < SIGN, -1.0, 1.0).astype(np.float32)
    return {
        "w1t": np.ascontiguousarray(W1.T),                           # [D, 2D]
        "w1tb": np.ascontiguousarray(W1.T).astype(bf16),             # [D, 2D]
        "w2t": np.ascontiguousarray(W2.T).astype(bf16),              # [2D, D]
        "w2sgn": np.ascontiguousarray(W2 * sign[:, None]).astype(bf16),
        "w1sgn": np.ascontiguousarray(W1 * sign[None, :]).astype(bf16),
        "w2t2": np.ascontiguousarray(-2.0 * W2.T).astype(bf16),
        "b1d": np.ascontiguousarray(b1.reshape(2, 128).T),           # [128, 2]
        "b2d": np.ascontiguousarray(b2.reshape(128, 1)),             # [128, 1]
        "idn": np.eye(128, dtype=np.float32),
    }


def _run(inp_np, W1, b1, W2, b2, trace=False):
    from concourse.bass_utils import run_bass_kernel_spmd

    nc = _get_nc(N_CORE)
    wmap = _host_weights(W1, b1, W2, b2)
    in_maps = []
    for c in range(NCORES):
        m = dict(wmap)
        m["inp"] = np.ascontiguousarray(
            inp_np[c * N_CORE:(c + 1) * N_CORE], np.float32)
        in_maps.append(m)
    res = run_bass_kernel_spmd(nc, in_maps, list(range(NCORES)), trace=trace)
    out = np.concatenate([r["out"] for r in res.results], axis=0)
    return out, res


def kernel(t=None, input_=None, W1=None, b1=None, W2=None, b2=None, **kw):
    inp_np = np.ascontiguousarray(np.asarray(input_, np.float32))
    trace = bool(int(os.environ.get("KERNEL_TRACE", "0")))
    out, _ = _run(inp_np, W1, b1, W2, b2, trace=trace)
    return out


def run_traced(inputs):
    """Returns (out, exec_time_ns, trace_path). Used by test.py."""
    inp_np = np.ascontiguousarray(np.asarray(inputs["input_"], np.float32))
    out, res = _run(inp_np, inputs["W1"], inputs["b1"], inputs["W2"],
                    inputs["b2"], trace=True)
    trace_path = None
    if res.instructions_and_trace is not None:
        trace_path = res.instructions_and_trace[1]
    return out, res.exec_time_ns, trace_path



# revision 16
# speedup vs baseline: 1.3069x; 1.3069x over previous
"""Trainium2 Bass kernel for nn_Connection_v5 (geodesic-spray-style RHS).

Math (per sample n, D=128, 2D=256):
    x = input_[:, :D], v = input_[:, D:]
    z1 = x @ W1.T + b1            [2D]
    h  = relu(z1), mask = z1 > 0  [2D]
    s  = sigmoid(h @ W2.T + b2)   [D]
    sign_j = -1 if j < 4 else 1
    g  = (s + 0.618) * sign;  jac[i,j] = sign_i s_i(1-s_i) * (W2 (mask*W1))[i,j]
    dv[j] = -1/g_j * sum_i v_i^2 jac[i,j] + 2 v_j / g_j * sum_i v_i jac[j,i]
    out = [v, dv]

Folded form used here (signs/scales pushed into host-precomputed weights):
    nsps = (s-1)*s            (= -s(1-s))
    gr   = 1/(s+0.618)
    [z1 | u] = W1b @ [x | v]  (single bf16 matmul, fused M1+M3)
    mu   = (h>0) * u
    p    = v * nsps ; wt = p * v
    at   = wt @ (sign_i*W2) ; am = (h>0) * at
    At   = am @ (W1*sign_j) ; Ct = mu @ (-2*W2.T)
    dv   = gr * (At + p*Ct)

Sharding: pure data-parallel over N=8192 across 8 cores (1024 rows each);
weights replicated. On-chip layout is feature-major [feat, n]; sample-major
<->feature-major conversion via PE transposes with an identity matrix.
Precision: all matmuls bf16 (fp32 PSUM accumulate); the relu mask comes from
the bf16-input z1 (measured ~7e-3 output rel err vs the 2e-2 gate).
Elementwise combine in fp32.
"""

import os
import numpy as np

D = 128
TWO_D = 256
N_TOTAL = 8192
NCORES = 8
N_CORE = N_TOTAL // NCORES  # 1024
NF = 256                    # samples per pipeline chunk (matmul moving dim)
CONST = 0.618
SIGN = 4
DEPTH = 2                   # backB(c - DEPTH + 1) emission lag

_CACHE = {}


def _build(n_core=N_CORE):
    """Build + compile the per-core Bass module (cached)."""
    from contextlib import ExitStack

    import concourse.bacc as bacc
    import concourse.mybir as mybir
    import concourse.tile as tile

    f32 = mybir.dt.float32
    bf16 = mybir.dt.bfloat16
    Act = mybir.ActivationFunctionType
    Op = mybir.AluOpType

    nchunk = n_core // NF
    nb = NF // 128  # 128-row blocks per chunk

    nc = bacc.Bacc("TRN2", target_bir_lowering=False, debug=False,
                   num_devices=NCORES)

    inp = nc.dram_tensor("inp", [n_core, TWO_D], f32, kind="ExternalInput").ap()
    w1b = nc.dram_tensor("w1b", [D, TWO_D], bf16, kind="ExternalInput").ap()
    w2t = nc.dram_tensor("w2t", [TWO_D, D], bf16, kind="ExternalInput").ap()
    w2sgn = nc.dram_tensor("w2sgn", [D, TWO_D], bf16,
                           kind="ExternalInput").ap()
    w1sgn = nc.dram_tensor("w1sgn", [TWO_D, D], bf16,
                           kind="ExternalInput").ap()
    w2t2 = nc.dram_tensor("w2t2", [TWO_D, D], bf16, kind="ExternalInput").ap()
    b1d = nc.dram_tensor("b1d", [D, 2], f32, kind="ExternalInput").ap()
    b2d = nc.dram_tensor("b2d", [D, 1], f32, kind="ExternalInput").ap()
    c618 = nc.dram_tensor("c618", [D, 1], f32, kind="ExternalInput").ap()
    idn = nc.dram_tensor("idn", [128, 128], f32, kind="ExternalInput").ap()
    out = nc.dram_tensor("out", [n_core, TWO_D], f32, kind="ExternalOutput").ap()

    with tile.TileContext(nc) as tc:
        with ExitStack() as ctx:
            singles = ctx.enter_context(tc.tile_pool(name="singles", bufs=1))
            io = ctx.enter_context(tc.tile_pool(name="io", bufs=4))
            acts = ctx.enter_context(tc.tile_pool(name="acts", bufs=4))
            psum = ctx.enter_context(
                tc.tile_pool(name="psum", bufs=1, space="PSUM"))

            # identity first (the transposes need it immediately); all other
            # weights go via SWDGE (gpsimd) so the Sync HWDGE queue is free
            # for the first input tiles.
            sb_id = singles.tile([128, 128], f32, name="sb_id")
            nc.sync.dma_start(out=sb_id, in_=idn)
            sb_b1 = singles.tile([128, 2], f32, name="sb_b1")
            nc.gpsimd.dma_start(out=sb_b1, in_=b1d)
            sb_b2 = singles.tile([128, 1], f32, name="sb_b2")
            nc.gpsimd.dma_start(out=sb_b2, in_=b2d)
            sb_c618 = singles.tile([128, 1], f32, name="sb_c618")
            nc.gpsimd.dma_start(out=sb_c618, in_=c618)
            # prime the ACT function tables (Relu/Sigmoid/Copy) with dummy
            # [128,1] ops so the ~1.3us ACT_TABLE_LOADs overlap the DMAs
            # instead of blocking the first real activation.
            warm = singles.tile([128, 1], f32, name="warm")
            nc.scalar.activation(out=warm, in_=sb_id[:, 0:1],
                                 func=Act.Relu, bias=sb_b2[:, 0:1], scale=1.0)
            nc.scalar.activation(out=warm, in_=sb_id[:, 0:1],
                                 func=Act.Sigmoid, bias=sb_b2[:, 0:1],
                                 scale=1.0)
            nc.scalar.activation(out=warm, in_=sb_id[:, 0:1],
                                 func=Act.Identity, bias=sb_b2[:, 0:1],
                                 scale=1.0)
            sb_w1b = singles.tile([128, TWO_D], bf16, name="sb_w1b")
            nc.gpsimd.dma_start(out=sb_w1b, in_=w1b)
            sb_w2t = singles.tile([128, 2, D], bf16, name="sb_w2t")
            nc.gpsimd.dma_start(out=sb_w2t,
                                in_=w2t.rearrange("(c p) m -> p c m", p=128))
            sb_w2sgn = singles.tile([128, TWO_D], bf16, name="sb_w2sgn")
            nc.gpsimd.dma_start(out=sb_w2sgn, in_=w2sgn)
            sb_w1sgn = singles.tile([128, 2, D], bf16, name="sb_w1sgn")
            nc.gpsimd.dma_start(out=sb_w1sgn,
                                in_=w1sgn.rearrange("(c p) m -> p c m", p=128))
            sb_w2t2 = singles.tile([128, 2, D], bf16, name="sb_w2t2")
            nc.gpsimd.dma_start(out=sb_w2t2,
                                in_=w2t2.rearrange("(c p) m -> p c m", p=128))

            inp_v = inp.rearrange("(c b p) d -> c p b d", p=128, b=nb)
            outd_v = out[:, D:TWO_D].rearrange("(c b p) d -> c p b d",
                                               p=128, b=nb)
            outv_v = out[:, 0:D].rearrange("(c b p) d -> c p b d",
                                           p=128, b=nb)

            state = {}

            def front_in(c):
                """DMA in, v passthrough, transposes, copies, fused M1+M3."""
                inb = io.tile([128, nb, TWO_D], f32, tag="inb", name="inb")
                nc.sync.dma_start(out=inb, in_=inp_v[c])
                # v passthrough straight from SBUF (input already on-chip)
                nc.sync.dma_start(out=outv_v[c], in_=inb[:, :, D:TWO_D])

                # sample-major -> feature-major via PE transposes
                ps_tr = psum.tile([128, 2, NF], f32, tag="tp", bufs=2,
                                  name="ps_tr")
                for b in range(nb):
                    nc.tensor.transpose(ps_tr[:, 0, 128 * b:128 * (b + 1)],
                                        inb[:, b, 0:D], sb_id)
                    nc.tensor.transpose(ps_tr[:, 1, 128 * b:128 * (b + 1)],
                                        inb[:, b, D:TWO_D], sb_id)
                # bf16 [x | v] for the fused M1+M3 moving operand
                xvb = acts.tile([128, 2, NF], bf16, tag="xvb", name="xvb")
                nc.scalar.copy(out=xvb, in_=ps_tr)
                # fp32 v for the elementwise chain
                vT = acts.tile([128, NF], f32, tag="vT", name="vT")
                nc.vector.tensor_scalar_add(vT, ps_tr[:, 1, :], 0.0)

                # fused M1+M3: [z1 | u]^T = W1b @ [x | v]^T   (512 moving cols)
                ps_uz = psum.tile([128, 2, 2 * NF], f32, tag="uz", bufs=1,
                                  name="ps_uz")
                xvf = xvb.rearrange("p a n -> p (a n)")
                for k in range(2):
                    nc.tensor.matmul(ps_uz[:, k, :],
                                     sb_w1b[:, 128 * k:128 * (k + 1)], xvf,
                                     start=True, stop=True)
                state[c] = dict(inb=inb, ps_tr=ps_tr, ps_uz=ps_uz, vT=vT)

            def front_main(c):
                """relu, mask*u, M2, sigmoid, gr/nsps."""
                st = state[c]
                ps_uz, vT = st.pop("ps_uz"), st["vT"]

                h = acts.tile([128, 2, NF], bf16, tag="h", name="h")
                for k in range(2):
                    nc.scalar.activation(out=h[:, k, :], in_=ps_uz[:, k, 0:NF],
                                         func=Act.Relu,
                                         bias=sb_b1[:, k:k + 1], scale=1.0)
                # mu = (h > 0) * u, u read straight from the M13 PSUM
                mu = acts.tile([128, 2, NF], bf16, tag="mu", name="mu")
                nc.vector.scalar_tensor_tensor(
                    out=mu, in0=h, scalar=0.0,
                    in1=ps_uz[:, :, NF:2 * NF],
                    op0=Op.is_gt, op1=Op.mult)

                # M2: z2 = W2 @ h (accumulate over the two k-chunks)
                ps_z2 = psum.tile([128, NF], f32, tag="z2", bufs=1,
                                  name="ps_z2")
                for k in range(2):
                    nc.tensor.matmul(ps_z2, sb_w2t[:, k, :], h[:, k, :],
                                     start=(k == 0), stop=(k == 1))
                s = acts.tile([128, NF], f32, tag="s", name="s")
                nc.scalar.activation(out=s, in_=ps_z2, func=Act.Sigmoid,
                                     bias=sb_b2[:, 0:1], scale=1.0)

                nsps = acts.tile([128, NF], f32, tag="nsps", name="nsps")
                nc.vector.scalar_tensor_tensor(out=nsps, in0=s, scalar=-1.0,
                                               in1=s, op0=Op.add, op1=Op.mult)
                gs = acts.tile([128, NF], f32, tag="gs", name="gs")
                nc.scalar.activation(out=gs, in_=s, func=Act.Identity,
                                     bias=sb_c618[:, 0:1], scale=1.0)
                gr = acts.tile([128, NF], f32, tag="gr", name="gr")
                nc.vector.reciprocal_approx_fast(out=gr, in_=gs)
                st.update(h=h, mu=mu, gr=gr, nsps=nsps)

            def backA(c):
                """p/wt/vq elementwise, M4, am."""
                st = state[c]
                vT, h = st.pop("vT"), st.pop("h")
                gr, nsps = st["gr"], st["nsps"]

                p = acts.tile([128, NF], f32, tag="p", name="p")
                nc.gpsimd.tensor_tensor(p, vT, nsps, Op.mult)
                wt = acts.tile([128, NF], bf16, tag="wt", name="wt")
                nc.gpsimd.tensor_tensor(wt, p, vT, Op.mult)

                # M4: at^T, contraction over i with (sign_i*W2)
                ps_a = psum.tile([128, 2, NF], f32, tag="a", bufs=1,
                                 name="ps_a")
                for k in range(2):
                    nc.tensor.matmul(ps_a[:, k, :],
                                     sb_w2sgn[:, 128 * k:128 * (k + 1)],
                                     wt, start=True, stop=True)
                am = acts.tile([128, 2, NF], bf16, tag="am", name="am")
                nc.vector.scalar_tensor_tensor(
                    out=am, in0=h, scalar=0.0, in1=ps_a,
                    op0=Op.is_gt, op1=Op.mult)
                st.update(am=am, p=p)

            def backB(c):
                st = state.pop(c)
                gr, p, mu, am = st["gr"], st["p"], st["mu"], st["am"]

                # M6: Ct = mu @ (-2*W2.T) first (tpc consumes it);
                # M5: At = am @ (W1*sign_j)
                ps_AC = psum.tile([128, 2, NF], f32, tag="AC", bufs=1,
                                  name="ps_AC")
                for k in range(2):
                    nc.tensor.matmul(ps_AC[:, 1, :], sb_w2t2[:, k, :],
                                     mu[:, k, :],
                                     start=(k == 0), stop=(k == 1))
                for k in range(2):
                    nc.tensor.matmul(ps_AC[:, 0, :], sb_w1sgn[:, k, :],
                                     am[:, k, :],
                                     start=(k == 0), stop=(k == 1))

                # dv = gr * (At + p*Ct)
                tpc = acts.tile([128, NF], f32, tag="tpc", name="tpc")
                nc.vector.tensor_tensor(tpc, p, ps_AC[:, 1, :], Op.mult)
                sm = acts.tile([128, NF], f32, tag="sm", name="sm")
                nc.vector.tensor_tensor(sm, ps_AC[:, 0, :], tpc, Op.add)
                dvT = acts.tile([128, NF], f32, tag="dvT", name="dvT")
                nc.gpsimd.tensor_tensor(dvT, gr, sm, Op.mult)

                # feature-major -> sample-major and store
                ps_dv = psum.tile([128, NF], f32, tag="tp", bufs=2,
                                  name="ps_dv")
                for b in range(nb):
                    nc.tensor.transpose(ps_dv[:, 128 * b:128 * (b + 1)],
                                        dvT[:, 128 * b:128 * (b + 1)], sb_id)
                ob = io.tile([128, nb, D], f32, tag="ob", name="ob")
                nc.scalar.copy(out=ob, in_=ps_dv.rearrange(
                    "p (b d) -> p b d", b=nb))
                nc.sync.dma_start(out=outd_v[c], in_=ob)

            lag = DEPTH - 1
            for c in range(nchunk):
                front_in(c)
                if c >= 1:
                    backA(c - 1)
                front_main(c)
                if c >= lag:
                    backB(c - lag)
            backA(nchunk - 1)
            for c in range(nchunk - lag, nchunk):
                backB(c)

    nc.compile()
    return nc


def _get_nc(n_core=N_CORE):
    key = ("nc", n_core)
    if key not in _CACHE:
        _CACHE[key] = _build(n_core)
    return _CACHE[key]


def _host_weights(W1, b1, W2, b2):
    import ml_dtypes

    W1 = np.asarray(W1, np.float32)
    b1 = np.asarray(b1, np.float32)
    W2 = np.asarray(W2, np.float32)
    b2 = np.asarray(b2, np.float32)
    bf16 = ml_dtypes.bfloat16
    sign = np.where(np.arange(D) < SIGN, -1.0, 1.0).astype(np.float32)
    b1r = np.ascontiguousarray(b1.reshape(2, 128).T)                 # [128, 2]
    return {
        "w1b": np.ascontiguousarray(W1.T).astype(bf16),              # [D, 2D]
        "w2t": np.ascontiguousarray(W2.T).astype(bf16),              # [2D, D]
        "w2sgn": np.ascontiguousarray(W2 * sign[:, None]).astype(bf16),
        "w1sgn": np.ascontiguousarray(W1 * sign[None, :]).astype(bf16),
        "w2t2": np.ascontiguousarray(-2.0 * W2.T).astype(bf16),
        "b1d": b1r,
        "b2d": np.ascontiguousarray(b2.reshape(128, 1)),             # [128, 1]
        "c618": np.full((128, 1), CONST, dtype=np.float32),
        "idn": np.eye(128, dtype=np.float32),
    }


def _run(inp_np, W1, b1, W2, b2, trace=False):
    from concourse.bass_utils import run_bass_kernel_spmd

    nc = _get_nc(N_CORE)
    wmap = _host_weights(W1, b1, W2, b2)
    in_maps = []
    for c in range(NCORES):
        m = dict(wmap)
        m["inp"] = np.ascontiguousarray(
            inp_np[c * N_CORE:(c + 1) * N_CORE], np.float32)
        in_maps.append(m)
    res = run_bass_kernel_spmd(nc, in_maps, list(range(NCORES)), trace=trace)
    out = np.concatenate([r["out"] for r in res.results], axis=0)
    return out, res


def kernel(t=None, input_=None, W1=None, b1=None, W2=None, b2=None, **kw):
    inp_np = np.ascontiguousarray(np.asarray(input_, np.float32))
    trace = bool(int(os.environ.get("KERNEL_TRACE", "0")))
    out, _ = _run(inp_np, W1, b1, W2, b2, trace=trace)
    return out


def run_traced(inputs):
    """Returns (out, exec_time_ns, trace_path). Used by test.py."""
    inp_np = np.ascontiguousarray(np.asarray(inputs["input_"], np.float32))
    out, res = _run(inp_np, inputs["W1"], inputs["b1"], inputs["W2"],
                    inputs["b2"], trace=True)
    trace_path = None
    if res.instructions_and_trace is not None:
        trace_path = res.instructions_and_trace[1]
    return out, res.exec_time_ns, trace_path


# revision 18
# speedup vs baseline: 1.3800x; 1.0559x over previous
"""Trainium2 Bass kernel for nn_Connection_v5 (geodesic-spray-style RHS).

Math (per sample n, D=128, 2D=256):
    x = input_[:, :D], v = input_[:, D:]
    z1 = x @ W1.T + b1            [2D]
    h  = relu(z1), mask = z1 > 0  [2D]
    s  = sigmoid(h @ W2.T + b2)   [D]
    sign_j = -1 if j < 4 else 1
    g  = (s + 0.618) * sign;  jac[i,j] = sign_i s_i(1-s_i) * (W2 (mask*W1))[i,j]
    dv[j] = -1/g_j * sum_i v_i^2 jac[i,j] + 2 v_j / g_j * sum_i v_i jac[j,i]
    out = [v, dv]

Folded form used here (signs/scales pushed into host-precomputed weights):
    nsps = (s-1)*s            (= -s(1-s))
    gr   = 1/(s+0.618)
    [z1 | u] = W1b @ [x | v]  (single bf16 matmul, fused M1+M3)
    mu   = (h>0) * u
    v2   = v*v ; wt = v2*nsps ; p = v*nsps
    at   = wt @ (sign_i*W2) ; am = (h>0) * at
    At   = am @ (W1*sign_j) ; Ct = mu @ (-2*W2.T)
    dv   = gr * (At + p*Ct)

Sharding: pure data-parallel over N=8192 across 8 cores (1024 rows each);
weights replicated.  On-chip layout is feature-major [feat, n]; sample-major
<->feature-major conversion via PE transposes with an identity matrix.
All matmuls bf16 (fp32 PSUM accumulate); the relu mask comes from the
bf16-input z1 (measured ~7e-3 output rel err vs the 2e-2 gate).  All
weights arrive in two packed DMAs on the sync HWDGE queue so compute can
start ~3us in (SWDGE weight loads were costing ~8us of fill).
"""

import os
import numpy as np

D = 128
TWO_D = 256
N_TOTAL = 8192
NCORES = 8
N_CORE = N_TOTAL // NCORES  # 1024
NF = 256                    # samples per pipeline chunk (matmul moving dim)
CONST = 0.618
SIGN = 4
DEPTH = 3                   # backB(c - DEPTH + 1) emission lag

_CACHE = {}

# bwall column layout (bf16): [w1b | w2t | w2sgn | w1sgn | w2t2 | idnb]
_W1B, _W2T, _W2SGN, _W1SGN, _W2T2, _IDNB = (
    0, TWO_D, 2 * TWO_D, 3 * TWO_D, 4 * TWO_D, 5 * TWO_D)
_BWALL = 5 * TWO_D + 128
# fwall column layout (f32): [idn | b1(2) | b2 | c618]
_FWALL = 128 + 4


def _build(n_core=N_CORE):
    """Build + compile the per-core Bass module (cached)."""
    from contextlib import ExitStack

    import concourse.bacc as bacc
    import concourse.mybir as mybir
    import concourse.tile as tile

    f32 = mybir.dt.float32
    bf16 = mybir.dt.bfloat16
    Act = mybir.ActivationFunctionType
    Op = mybir.AluOpType

    nchunk = n_core // NF
    nb = NF // 128  # 128-row blocks per chunk

    nc = bacc.Bacc("TRN2", target_bir_lowering=False, debug=False,
                   num_devices=NCORES)

    inp = nc.dram_tensor("inp", [n_core, TWO_D], f32, kind="ExternalInput").ap()
    fwall = nc.dram_tensor("fwall", [128, _FWALL], f32,
                           kind="ExternalInput").ap()
    bwall = nc.dram_tensor("bwall", [128, _BWALL], bf16,
                           kind="ExternalInput").ap()
    out = nc.dram_tensor("out", [n_core, TWO_D], f32, kind="ExternalOutput").ap()

    with tile.TileContext(nc) as tc:
        with ExitStack() as ctx:
            singles = ctx.enter_context(tc.tile_pool(name="singles", bufs=1))
            io = ctx.enter_context(tc.tile_pool(name="io", bufs=4))
            acts = ctx.enter_context(tc.tile_pool(name="acts", bufs=4))
            psum = ctx.enter_context(
                tc.tile_pool(name="psum", bufs=1, space="PSUM"))

            # all constants/weights in two HWDGE DMAs, on-chip by ~2.5us
            sb_f = singles.tile([128, _FWALL], f32, name="sb_f")
            nc.sync.dma_start(out=sb_f, in_=fwall)
            sb_w = singles.tile([128, _BWALL], bf16, name="sb_w")
            nc.sync.dma_start(out=sb_w, in_=bwall)
            sb_id = sb_f[:, 0:128]
            sb_b1 = sb_f[:, 128:130]
            sb_b2 = sb_f[:, 130:131]
            sb_c618 = sb_f[:, 131:132]
            sb_idb = sb_w[:, _IDNB:_IDNB + 128]

            # prime the ACT function tables (Relu/Sigmoid/Identity/Copy) so
            # the ~1.3us ACT_TABLE_LOADs overlap the DMAs instead of blocking
            # the first real activation.
            warm = singles.tile([128, 1], f32, name="warm")
            nc.scalar.activation(out=warm, in_=sb_f[:, 0:1],
                                 func=Act.Relu, bias=sb_b2[:, 0:1], scale=1.0)
            nc.scalar.activation(out=warm, in_=sb_f[:, 0:1],
                                 func=Act.Sigmoid, bias=sb_b2[:, 0:1],
                                 scale=1.0)
            nc.scalar.activation(out=warm, in_=sb_f[:, 0:1],
                                 func=Act.Identity, bias=sb_b2[:, 0:1],
                                 scale=1.0)
            nc.scalar.copy(out=warm, in_=sb_f[:, 0:1])

            inp_v = inp.rearrange("(c b p) d -> c p b d", p=128, b=nb)
            outd_v = out[:, D:TWO_D].rearrange("(c b p) d -> c p b d",
                                               p=128, b=nb)
            outv_v = out[:, 0:D].rearrange("(c b p) d -> c p b d",
                                           p=128, b=nb)

            state = {}

            def front_in(c):
                """DMA in, v passthrough, transposes, copies, fused M1+M3."""
                inb = io.tile([128, nb, TWO_D], f32, tag="inb", name="inb")
                nc.sync.dma_start(out=inb, in_=inp_v[c])
                # v passthrough straight from SBUF (input already on-chip)
                nc.sync.dma_start(out=outv_v[c], in_=inb[:, :, D:TWO_D])

                # sample-major -> feature-major via PE transposes
                ps_tr = psum.tile([128, 2, NF], f32, tag="tp", bufs=2,
                                  name="ps_tr")
                for b in range(nb):
                    nc.tensor.transpose(ps_tr[:, 0, 128 * b:128 * (b + 1)],
                                        inb[:, b, 0:D], sb_id)
                    nc.tensor.transpose(ps_tr[:, 1, 128 * b:128 * (b + 1)],
                                        inb[:, b, D:TWO_D], sb_id)
                # bf16 [x | v] for the fused M1+M3 moving operand
                xvb = acts.tile([128, 2, NF], bf16, tag="xvb", name="xvb")
                nc.scalar.copy(out=xvb, in_=ps_tr)
                # fp32 v for the elementwise chain
                vT = acts.tile([128, NF], f32, tag="vT", name="vT")
                nc.vector.tensor_scalar_add(vT, ps_tr[:, 1, :], 0.0)
                v2 = acts.tile([128, NF], f32, tag="v2", name="v2")
                nc.gpsimd.tensor_tensor(v2, vT, vT, Op.mult)

                # fused M1+M3: [z1 | u]^T = W1b @ [x | v]^T  (512 moving cols)
                ps_uz = psum.tile([128, 2, 2 * NF], f32, tag="uz", bufs=1,
                                  name="ps_uz")
                xvf = xvb.rearrange("p a n -> p (a n)")
                for k in range(2):
                    nc.tensor.matmul(ps_uz[:, k, :],
                                     sb_w[:, _W1B + 128 * k:_W1B + 128 * (k + 1)],
                                     xvf, start=True, stop=True)
                state[c] = dict(ps_uz=ps_uz, vT=vT, v2=v2)

            def front_main(c):
                """relu, mask*u, M2, sigmoid, gr/nsps."""
                st = state[c]
                ps_uz = st.pop("ps_uz")

                h = acts.tile([128, 2, NF], bf16, tag="h", name="h")
                for k in range(2):
                    nc.scalar.activation(out=h[:, k, :], in_=ps_uz[:, k, 0:NF],
                                         func=Act.Relu,
                                         bias=sb_b1[:, k:k + 1], scale=1.0)
                # mu = (h > 0) * u, u read straight from the M13 PSUM
                mu = acts.tile([128, 2, NF], bf16, tag="mu", name="mu")
                nc.vector.scalar_tensor_tensor(
                    out=mu, in0=h, scalar=0.0,
                    in1=ps_uz[:, :, NF:2 * NF],
                    op0=Op.is_gt, op1=Op.mult)

                # M2: z2 = W2 @ h (accumulate over the two k-chunks)
                ps_z2 = psum.tile([128, NF], f32, tag="z2", bufs=1,
                                  name="ps_z2")
                for k in range(2):
                    nc.tensor.matmul(
                        ps_z2, sb_w[:, _W2T + 128 * k:_W2T + 128 * (k + 1)],
                        h[:, k, :], start=(k == 0), stop=(k == 1))
                s = acts.tile([128, NF], f32, tag="s", name="s")
                nc.scalar.activation(out=s, in_=ps_z2, func=Act.Sigmoid,
                                     bias=sb_b2[:, 0:1], scale=1.0)

                nsps = acts.tile([128, NF], f32, tag="nsps", name="nsps")
                nc.vector.scalar_tensor_tensor(out=nsps, in0=s, scalar=-1.0,
                                               in1=s, op0=Op.add, op1=Op.mult)
                gs = acts.tile([128, NF], f32, tag="gs", name="gs")
                nc.scalar.activation(out=gs, in_=s, func=Act.Identity,
                                     bias=sb_c618[:, 0:1], scale=1.0)
                gr = acts.tile([128, NF], f32, tag="gr", name="gr")
                nc.vector.reciprocal_approx_fast(out=gr, in_=gs)
                st.update(h=h, mu=mu, gr=gr, nsps=nsps)

            def backA(c):
                """wt, M4, am."""
                st = state[c]
                h, v2 = st.pop("h"), st.pop("v2")
                nsps = st["nsps"]

                wt = acts.tile([128, NF], bf16, tag="wt", name="wt")
                nc.gpsimd.tensor_tensor(wt, v2, nsps, Op.mult)

                # M4: at^T, contraction over i with (sign_i*W2)
                ps_a = psum.tile([128, 2, NF], f32, tag="a", bufs=1,
                                 name="ps_a")
                for k in range(2):
                    nc.tensor.matmul(
                        ps_a[:, k, :],
                        sb_w[:, _W2SGN + 128 * k:_W2SGN + 128 * (k + 1)],
                        wt, start=True, stop=True)
                am = acts.tile([128, 2, NF], bf16, tag="am", name="am")
                nc.vector.scalar_tensor_tensor(
                    out=am, in0=h, scalar=0.0, in1=ps_a,
                    op0=Op.is_gt, op1=Op.mult)
                st.update(am=am)

            def backB(c):
                st = state.pop(c)
                gr, mu, am = st["gr"], st["mu"], st["am"]
                vT, nsps = st["vT"], st["nsps"]

                # p = v*nsps, needed only for the tail combine
                p = acts.tile([128, NF], f32, tag="p", name="p")
                nc.gpsimd.tensor_tensor(p, vT, nsps, Op.mult)

                # M6: Ct = mu @ (-2*W2.T) first (tpc consumes it);
                # M5: At = am @ (W1*sign_j)
                ps_AC = psum.tile([128, 2, NF], f32, tag="AC", bufs=1,
                                  name="ps_AC")
                for k in range(2):
                    nc.tensor.matmul(
                        ps_AC[:, 1, :],
                        sb_w[:, _W2T2 + 128 * k:_W2T2 + 128 * (k + 1)],
                        mu[:, k, :], start=(k == 0), stop=(k == 1))
                for k in range(2):
                    nc.tensor.matmul(
                        ps_AC[:, 0, :],
                        sb_w[:, _W1SGN + 128 * k:_W1SGN + 128 * (k + 1)],
                        am[:, k, :], start=(k == 0), stop=(k == 1))

                # dv = gr * (At + p*Ct)
                tpc = acts.tile([128, NF], f32, tag="tpc", name="tpc")
                nc.vector.tensor_tensor(tpc, p, ps_AC[:, 1, :], Op.mult)
                sm = acts.tile([128, NF], f32, tag="sm", name="sm")
                nc.vector.tensor_tensor(sm, ps_AC[:, 0, :], tpc, Op.add)
                dvT = acts.tile([128, NF], bf16, tag="dvT", name="dvT")
                nc.gpsimd.tensor_tensor(dvT, gr, sm, Op.mult)

                # feature-major -> sample-major (bf16 transpose) and store
                ps_dv = psum.tile([128, NF], bf16, tag="tp", bufs=2,
                                  name="ps_dv")
                for b in range(nb):
                    nc.tensor.transpose(ps_dv[:, 128 * b:128 * (b + 1)],
                                        dvT[:, 128 * b:128 * (b + 1)], sb_idb)
                ob = io.tile([128, nb, D], f32, tag="ob", name="ob")
                nc.scalar.copy(out=ob, in_=ps_dv.rearrange(
                    "p (b d) -> p b d", b=nb))
                nc.sync.dma_start(out=outd_v[c], in_=ob)

            lag = DEPTH - 1
            for c in range(nchunk):
                front_in(c)
                if c >= 1:
                    backA(c - 1)
                front_main(c)
                if c >= lag:
                    backB(c - lag)
            backA(nchunk - 1)
            for c in range(max(0, nchunk - lag), nchunk):
                backB(c)

    nc.compile()
    return nc


def _get_nc(n_core=N_CORE):
    key = ("nc", n_core)
    if key not in _CACHE:
        _CACHE[key] = _build(n_core)
    return _CACHE[key]


def _pack_k(mat):
    """[2D, D] -> [128, 2*128] with the k-chunk partition packing the
    matmul stationary slices expect ([p, (c m)] where row = c*128+p)."""
    return np.ascontiguousarray(
        mat.reshape(2, 128, 128).transpose(1, 0, 2).reshape(128, 256))


def _host_weights(W1, b1, W2, b2):
    import ml_dtypes

    W1 = np.asarray(W1, np.float32)
    b1 = np.asarray(b1, np.float32)
    W2 = np.asarray(W2, np.float32)
    b2 = np.asarray(b2, np.float32)
    bf16 = ml_dtypes.bfloat16
    sign = np.where(np.arange(D) < SIGN, -1.0, 1.0).astype(np.float32)

    bwall = np.zeros((128, _BWALL), np.float32)
    bwall[:, _W1B:_W1B + TWO_D] = W1.T                       # [D, 2D]
    bwall[:, _W2T:_W2T + TWO_D] = _pack_k(W2.T.copy())       # [2D, D] packed
    bwall[:, _W2SGN:_W2SGN + TWO_D] = W2 * sign[:, None]     # [D, 2D]
    bwall[:, _W1SGN:_W1SGN + TWO_D] = _pack_k(W1 * sign[None, :])
    bwall[:, _W2T2:_W2T2 + TWO_D] = _pack_k(-2.0 * W2.T.copy())
    bwall[:, _IDNB:_IDNB + 128] = np.eye(128, dtype=np.float32)

    fwall = np.zeros((128, _FWALL), np.float32)
    fwall[:, 0:128] = np.eye(128, dtype=np.float32)
    fwall[:, 128:130] = b1.reshape(2, 128).T
    fwall[:, 130] = b2
    fwall[:, 131] = CONST

    return {
        "fwall": np.ascontiguousarray(fwall),
        "bwall": np.ascontiguousarray(bwall).astype(bf16),
    }


def _run(inp_np, W1, b1, W2, b2, trace=False):
    from concourse.bass_utils import run_bass_kernel_spmd

    nc = _get_nc(N_CORE)
    wmap = _host_weights(W1, b1, W2, b2)
    in_maps = []
    for c in range(NCORES):
        m = dict(wmap)
        m["inp"] = np.ascontiguousarray(
            inp_np[c * N_CORE:(c + 1) * N_CORE], np.float32)
        in_maps.append(m)
    res = run_bass_kernel_spmd(nc, in_maps, list(range(NCORES)), trace=trace)
    out = np.concatenate([r["out"] for r in res.results], axis=0)
    return out, res


def kernel(t=None, input_=None, W1=None, b1=None, W2=None, b2=None, **kw):
    inp_np = np.ascontiguousarray(np.asarray(input_, np.float32))
    trace = bool(int(os.environ.get("KERNEL_TRACE", "0")))
    out, _ = _run(inp_np, W1, b1, W2, b2, trace=trace)
    return out


def run_traced(inputs):
    """Returns (out, exec_time_ns, trace_path). Used by test.py."""
    inp_np = np.ascontiguousarray(np.asarray(inputs["input_"], np.float32))
    out, res = _run(inp_np, inputs["W1"], inputs["b1"], inputs["W2"],
                    inputs["b2"], trace=True)
    trace_path = None
    if res.instructions_and_trace is not None:
        trace_path = res.instructions_and_trace[1]
    return out, res.exec_time_ns, trace_path
